# revision 14
# baseline (speedup 1.0000x reference)
"""Trainium2 Bass kernel for a dense transformer encoder layer.

Shapes: B=4, S=2048, D=1024, H=16 heads x DH=64.
Sharding: sequence-parallel over 8 cores — core c handles batch c//2,
query rows (c%2)*1024..+1024; K/V computed for the full batch sequence
on both cores of a pair (duplicated, no collectives).

Layout strategy: all on-chip activations are kept transposed
[features -> partitions (tiles of 128), tokens -> free dim] so that every
matmul contraction lands on the partition axis without any on-chip
transposes.  The host pre-transposes x and post-transposes the output.

Precision: Q/O/FFN projections run fp32r (full PE rate at N=512, ~1.5e-4
matmul rel err); K/V projections and attention run bf16 with fp32 PSUM
accumulation.  Softmax skips max-subtraction (scores ~ N(0,1) after the
1/sqrt(dh) scale, folded into Exp's affine); the softmax denominator
comes from a ones-column appended to V, so attn @ [V|1] yields context
and sum-of-exponentials in one accumulation.

Schedule: Q and V projections run up front; then a per-head-pair
software pipeline interleaves the K projection, scores matmuls, Exp on
ScalarE, and context matmuls so TensorE and ScalarE overlap instead of
alternating (keeps the PE HAM clock warm).

SBUF is a two-sided stack allocator: long-lived tensors go on the left
stack, phase-transient ones on the right, freed in LIFO order per side.
"""

import numpy as np

import concourse.bass as bass
import concourse.mybir as mybir
import concourse.tile as tile
from concourse import bacc
from concourse.bass_utils import run_bass_kernel_spmd

F32 = mybir.dt.float32
F32R = mybir.dt.float32r
BF16 = mybir.dt.bfloat16
AF = mybir.ActivationFunctionType

B, S, D, H = 4, 2048, 1024, 16
DH = D // H          # 64
NT = D // 128        # 8 feature tiles
KTT = S // 128       # 16 key-token tiles
MQ = S // 2          # 1024 local query rows per core
QC = MQ // 512       # 2 query chunks of 512
EPS = 1e-6


def build_nc():
    nc = bacc.Bacc("TRN2", target_bir_lowering=False, debug=False, num_devices=8)

    xkvT = nc.declare_dram_parameter("xkvT", [D, S], F32, isOutput=False)
    xqT = nc.declare_dram_parameter("xqT", [D, MQ], F32, isOutput=False)
    w_par = {}
    b_par = {}
    for nm in ("wq", "wk", "wv", "wo", "wf"):
        w_par[nm] = nc.declare_dram_parameter(nm, [D, D], F32, isOutput=False)
    for nm in ("bq", "bk", "bv", "bo", "bf", "g1", "b1", "g2", "b2"):
        b_par[nm] = nc.declare_dram_parameter(nm, [D], F32, isOutput=False)
    outT = nc.declare_dram_parameter("outT", [D, MQ], F32, isOutput=True)

    with tile.TileContext(nc) as tc:
        keepalive = build_body(nc, tc, xkvT, xqT, w_par, b_par, outT)

    nc.compile()
    del keepalive
    return nc


def load_wcol(nc, pool, w, nt, dt=F32R, tag="wcol"):
    """weight column block w[:, nt*128:(nt+1)*128] -> SBUF [128, NT, 128]"""
    t = pool.tile([128, NT, 128], dt, name=f"wcol_{w.name}_{nt}", tag=tag)
    src = w[:, nt * 128:(nt + 1) * 128].rearrange("(t p) n -> p t n", p=128)
    nc.sync.dma_start(out=t, in_=src.bitcast(dt))
    return t


def bcast_row(nc, out_ap, row_ap, nparts):
    """replicate a [1, N] SBUF row across nparts partitions via DMA"""
    inner = list(row_ap.ap[-1])
    src = bass.AP(tensor=row_ap.tensor, offset=row_ap.offset,
                  ap=[[1, 1], [0, nparts], inner])
    nc.sync.dma_start(out=out_ap, in_=src)


def build_body(nc, tc, xkvT, xqT, w_par, b_par, outT):
    f32 = F32

    # ---------------- constants / biases (whole-kernel, left stack) ----------
    final_frees = []  # popped in reverse at the end (left-stack LIFO)
    bias_sb = {}
    for nm in ("bq", "bk", "bo", "bf", "g1", "b1", "g2", "b2"):
        t, fr = tc.tile([128, NT], f32, name=f"bias_{nm}", side="left")
        nc.sync.dma_start(out=t, in_=b_par[nm][:].rearrange("(t p) -> p t", p=128))
        bias_sb[nm] = t
        final_frees.append(fr)
    # bv broadcast across partitions [128, D] (free-dim bias for natural-layout V)
    bvb, bvb_fr = tc.tile([128, D], f32, name="bvb", side="left")
    final_frees.append(bvb_fr)
    bv_ap = b_par["bv"][:]
    bv_bc = bass.AP(tensor=bv_ap.tensor, offset=bv_ap.offset, ap=[[0, 128], [1, D]])
    nc.sync.dma_start(out=bvb, in_=bv_bc)
    ones_sb, ones_fr = tc.tile([128, 1], F32R, name="ones_sb", side="left")
    final_frees.append(ones_fr)
    ones_f32, ones_f32_fr = tc.tile([128, 1], f32, name="ones_f32", side="left")
    final_frees.append(ones_f32_fr)
    nc.vector.memset(ones_f32, 1.0)
    nc.scalar.copy(out=ones_sb, in_=ones_f32)
    eps_sb, eps_fr = tc.tile([1, 1], f32, name="eps_sb", side="left")
    final_frees.append(eps_fr)
    nc.vector.memset(eps_sb, EPS)

    # ---------------- x loads: bf16 copy of xkv^T for K/V ---------------------
    xkv16, xkv16_free = tc.tile([128, NT, S], BF16, name="xkv16", side="left")
    xkv_f32, xkv_f32_free = tc.tile([128, NT, S], f32, name="xkv_f32", side="right")
    nc.sync.dma_start(out=xkv_f32,
                      in_=xkvT[:, :].rearrange("(t p) m -> p t m", p=128))
    for nt in range(NT):
        nc.vector.tensor_copy(out=xkv16[:, nt, :], in_=xkv_f32[:, nt, :])
    xkv_f32_free()

    # ======================= Q projection (fp32r) ============================
    QT, QT_free = tc.tile([128, NT, MQ], BF16, name="QT", side="left")
    with tc.tile_pool(name="qps", bufs=4, space="PSUM") as qps:
        xq_sb, xq_free = tc.tile([128, NT, MQ], F32R, name="xq_sb", side="right")
        nc.sync.dma_start(
            out=xq_sb,
            in_=xqT[:, :].rearrange("(t p) m -> p t m", p=128).bitcast(F32R))
        with tc.tile_pool(name="wqpool", bufs=2, side="left") as wqpool:
            for nt in range(NT):
                wc = load_wcol(nc, wqpool, w_par["wq"], nt)
                for qc in range(QC):
                    ps = qps.tile([128, 512], f32, name="ps_q", tag="p1")
                    for kt in range(NT):
                        nc.tensor.matmul(
                            ps, wc[:, kt, :],
                            xq_sb[:, kt, qc * 512:(qc + 1) * 512],
                            start=(kt == 0), stop=(kt == NT - 1))
                    nc.vector.tensor_scalar(
                        out=QT[:, nt, qc * 512:(qc + 1) * 512], in0=ps,
                        scalar1=bias_sb["bq"][:, nt:nt + 1], scalar2=None,
                        op0=mybir.AluOpType.add)
        xq_free()

    # ======================= V projection (bf16) -> VA [tok, head, 65] =======
    VA, VA_free = tc.tile([128, KTT, H, DH + 1], BF16, name="VA", side="left")
    with tc.tile_pool(name="vps", bufs=4, space="PSUM") as vps:
        with tc.tile_pool(name="wvpool", bufs=2, side="left") as wvpool:
            for dc in range(2):  # d-range halves of 512
                wv_f32 = wvpool.tile([128, NT, 512], f32,
                                     name=f"wv_f32_{dc}", tag="wvf")
                nc.sync.dma_start(
                    out=wv_f32,
                    in_=w_par["wv"][:, dc * 512:(dc + 1) * 512].rearrange(
                        "(t p) n -> p t n", p=128))
                wv16 = wvpool.tile([128, NT, 512], BF16,
                                   name=f"wv16_{dc}", tag="wv16")
                nc.vector.tensor_copy(out=wv16, in_=wv_f32)
                for tt in range(KTT):
                    ps = vps.tile([128, 512], f32, name="ps_v", tag="pv")
                    for kt in range(NT):
                        nc.tensor.matmul(
                            ps, xkv16[:, kt, tt * 128:(tt + 1) * 128],
                            wv16[:, kt, :],
                            start=(kt == 0), stop=(kt == NT - 1))
                    nc.vector.tensor_add(
                        out=VA[:, tt, dc * 8:(dc + 1) * 8, 0:DH],
                        in0=ps.rearrange("p (h d) -> p h d", h=8),
                        in1=bvb[:, dc * 512:(dc + 1) * 512].rearrange(
                            "p (h d) -> p h d", h=8))
            for tt in range(KTT):
                nc.vector.memset(VA[:, tt, :, DH:DH + 1], 1.0)

    # ======================= attention pipeline ==============================
    # per head-pair hp: K-projection for feature tile hp (heads 2hp, 2hp+1),
    # then scores -> exp -> ctx with a one-group software pipeline.
    ctxT, ctxT_free = tc.tile([128, NT, MQ], F32R, name="ctxT", side="right")

    with tc.tile_pool(name="wkpool", bufs=2, side="left") as wkpool, \
         tc.tile_pool(name="ktpool", bufs=2, side="left") as ktpool, \
         tc.tile_pool(name="exppool", bufs=4, side="left") as exppool, \
         tc.tile_pool(name="sepool", bufs=4, side="left") as sepool, \
         tc.tile_pool(name="bcpool", bufs=2, side="left") as bcpool, \
         tc.tile_pool(name="kps", bufs=2, space="PSUM") as kps, \
         tc.tile_pool(name="sps", bufs=2, space="PSUM") as sps, \
         tc.tile_pool(name="cps", bufs=2, space="PSUM") as cps:

        for hp in range(H // 2):
            h0, h1 = 2 * hp, 2 * hp + 1
            # ---- K projection for feature tile hp over the full sequence
            wk_f32 = wkpool.tile([128, NT, 128], f32, name="wk_f32", tag="wkf")
            nc.sync.dma_start(
                out=wk_f32,
                in_=w_par["wk"][:, hp * 128:(hp + 1) * 128].rearrange(
                    "(t p) n -> p t n", p=128))
            wk16 = wkpool.tile([128, NT, 128], BF16, name="wk16", tag="wk16")
            nc.vector.tensor_copy(out=wk16, in_=wk_f32)
            KTh = ktpool.tile([128, S], BF16, name="KTh", tag="kth")
            for mc in range(S // 512):
                ps = kps.tile([128, 512], f32, name="ps_k", tag="pk")
                for kt in range(NT):
                    nc.tensor.matmul(
                        ps, wk16[:, kt, :],
                        xkv16[:, kt, mc * 512:(mc + 1) * 512],
                        start=(kt == 0), stop=(kt == NT - 1))
                nc.vector.tensor_scalar(
                    out=KTh[:, mc * 512:(mc + 1) * 512], in0=ps,
                    scalar1=bias_sb["bk"][:, hp:hp + 1], scalar2=None,
                    op0=mybir.AluOpType.add)

            # ---- attention for heads (h0, h1)
            se0 = sepool.tile([1, MQ], f32, name="se0", tag="se")
            se1 = sepool.tile([1, MQ], f32, name="se1", tag="se")
            for qc in range(QC):
                qs = slice(qc * 512, (qc + 1) * 512)
                pc0 = cps.tile([128, 512], f32, name="pc0", tag="pc")
                pc1 = cps.tile([128, 512], f32, name="pc1", tag="pc")
                e_by_half = {}

                def emit_ctx(half, g):
                    e0g, e1g = e_by_half[half]
                    for j in range(2):
                        kt = half * 8 + g * 2 + j
                        nc.tensor.matmul(
                            pc0[0:DH + 1, :], VA[:, kt, h0, :],
                            e0g[:, g * 2 + j, :],
                            start=(kt == 0), stop=(kt == KTT - 1))
                        nc.tensor.matmul(
                            pc1[0:DH + 1, :], VA[:, kt, h1, :],
                            e1g[:, g * 2 + j, :],
                            start=(kt == 0), stop=(kt == KTT - 1))

                groups = [(half, g) for half in range(2) for g in range(4)]
                for idx, (half, g) in enumerate(groups):
                    if g == 0:
                        e_by_half[half] = (
                            exppool.tile([128, 8, 512], BF16, name="e0", tag="exp"),
                            exppool.tile([128, 8, 512], BF16, name="e1", tag="exp"))
                    ps0 = sps.tile([128, 2, 512], f32, name="ps0", tag="sc")
                    ps1 = sps.tile([128, 2, 512], f32, name="ps1", tag="sc")
                    for j in range(2):
                        kt = half * 8 + g * 2 + j
                        ks = slice(kt * 128, (kt + 1) * 128)
                        nc.tensor.matmul(
                            ps0[:, j, :], KTh[0:64, ks], QT[0:64, hp, qs],
                            start=True, stop=True, tile_position=(0, 0))
                        nc.tensor.matmul(
                            ps1[:, j, :], KTh[64:128, ks], QT[64:128, hp, qs],
                            start=True, stop=True, tile_position=(64, 0))
                    e0g, e1g = e_by_half[half]
                    nc.scalar.activation(
                        out=e0g[:, g * 2:(g + 1) * 2, :], in_=ps0,
                        func=AF.Exp, scale=1.0 / np.sqrt(DH))
                    nc.scalar.activation(
                        out=e1g[:, g * 2:(g + 1) * 2, :], in_=ps1,
                        func=AF.Exp, scale=1.0 / np.sqrt(DH))
                    if idx > 0:
                        emit_ctx(*groups[idx - 1])
                emit_ctx(*groups[-1])

                nc.vector.reciprocal(out=se0[0:1, qs], in_=pc0[DH:DH + 1, :])
                nc.vector.reciprocal(out=se1[0:1, qs], in_=pc1[DH:DH + 1, :])
                nc.vector.tensor_copy(out=ctxT[0:64, hp, qs], in_=pc0[0:DH, :])
                nc.vector.tensor_copy(out=ctxT[64:128, hp, qs], in_=pc1[0:DH, :])
            # normalize ctx by 1/sum_exp (per head, broadcast over DH partitions)
            bc = bcpool.tile([128, MQ], f32, name="bc", tag="bc")
            bcast_row(nc, bc[0:64, :], se0[0:1, :], 64)
            bcast_row(nc, bc[64:128, :], se1[0:1, :], 64)
            nc.vector.tensor_mul(out=ctxT[:, hp, :], in0=ctxT[:, hp, :], in1=bc)

    VA_free()
    QT_free()
    xkv16_free()

    # ======================= P3: output projection + residual ================
    y1, y1_free = tc.tile([128, NT, MQ], F32R, name="y1", side="left")
    xq2, xq2_free = tc.tile([128, NT, MQ], f32, name="xq2", side="right")
    nc.sync.dma_start(out=xq2, in_=xqT[:, :].rearrange("(t p) m -> p t m", p=128))

    with tc.tile_pool(name="wpool3", bufs=2, side="left") as wpool3, \
         tc.tile_pool(name="p3ps", bufs=4, space="PSUM") as p3ps:
        for nt in range(NT):
            wc = load_wcol(nc, wpool3, w_par["wo"], nt)
            for qc in range(QC):
                qs = slice(qc * 512, (qc + 1) * 512)
                ps = p3ps.tile([128, 512], f32, name="ps_o", tag="p3")
                for kt in range(NT):
                    nc.tensor.matmul(
                        ps, wc[:, kt, :], ctxT[:, kt, qs],
                        start=(kt == 0), stop=(kt == NT - 1))
                nc.vector.tensor_scalar(
                    out=y1[:, nt, qs], in0=ps,
                    scalar1=bias_sb["bo"][:, nt:nt + 1], scalar2=None,
                    op0=mybir.AluOpType.add)
                nc.vector.tensor_add(out=y1[:, nt, qs], in0=y1[:, nt, qs],
                                     in1=xq2[:, nt, qs])
    xq2_free()
    ctxT_free()

    # ======================= P4: LayerNorm 1 =================================
    n1, n1_free = tc.tile([128, NT, MQ], F32R, name="n1", side="right")
    layernorm(nc, tc, src=y1, dst=n1, g_sb=bias_sb["g1"], b_sb=bias_sb["b1"],
              ones_sb=ones_sb, eps_sb=eps_sb, label="ln1")
    y1_free()

    # ======================= P5: FFN ==========================================
    fnn, fnn_free = tc.tile([128, NT, MQ], F32R, name="fnn", side="left")
    with tc.tile_pool(name="wpool5", bufs=2, side="left") as wpool5, \
         tc.tile_pool(name="p5ps", bufs=4, space="PSUM") as p5ps:
        for nt in range(NT):
            wc = load_wcol(nc, wpool5, w_par["wf"], nt)
            for qc in range(QC):
                qs = slice(qc * 512, (qc + 1) * 512)
                ps = p5ps.tile([128, 512], f32, name="ps_f", tag="p5")
                for kt in range(NT):
                    nc.tensor.matmul(
                        ps, wc[:, kt, :], n1[:, kt, qs],
                        start=(kt == 0), stop=(kt == NT - 1))
                # fnn = relu(psum + bf)
                nc.scalar.activation(
                    out=fnn[:, nt, qs], in_=ps, func=AF.Relu,
                    bias=bias_sb["bf"][:, nt:nt + 1], scale=1.0)

    # y2 = fnn + n1 (overwrite fnn) — on GpSimd to offload VectorE
    for nt in range(NT):
        nc.gpsimd.tensor_add(out=fnn[:, nt, :], in0=fnn[:, nt, :], in1=n1[:, nt, :])
    n1_free()

    # ======================= P6: LayerNorm 2 -> output =======================
    with tc.tile_pool(name="outpool", bufs=2, side="left") as outpool:
        ab = ln_stats(nc, tc, src=fnn, ones_sb=ones_sb, eps_sb=eps_sb, label="ln2")
        for nt in range(NT):
            n2t = outpool.tile([128, MQ], f32, name="n2t", tag="n2")
            apply_ln(nc, n2t, fnn[:, nt, :], ab,
                     bias_sb["g2"][:, nt:nt + 1], bias_sb["b2"][:, nt:nt + 1])
            nc.sync.dma_start(out=outT[nt * 128:(nt + 1) * 128, :], in_=n2t)
        for fr in ab[2]:
            fr()

    # pop remaining left-stack singles in LIFO order
    fnn_free()
    for fr in reversed(final_frees):
        fr()
    return []


def ln_stats(nc, tc, src, ones_sb, eps_sb, label):
    """column sums of src and src^2 over all D partitions via ones-matmuls;
    returns (Abc, Bbc, frees): broadcast tiles with A=rstd, B=-mean*rstd."""
    A, A_free = tc.tile([1, MQ], F32, name=f"A_{label}", side="right")
    Bt, B_free = tc.tile([1, MQ], F32, name=f"B_{label}", side="right")
    m_sb, m_free = tc.tile([1, MQ], F32, name=f"m_{label}", side="right")
    with tc.tile_pool(name=f"sq_{label}", bufs=2, side="left") as sqpool, \
         tc.tile_pool(name=f"lnps_{label}", bufs=2, space="PSUM") as lnps:
        for qc in range(QC):
            qs = slice(qc * 512, (qc + 1) * 512)
            ps_sum = lnps.tile([128, 512], F32, name="ps_sum", tag="lnsum")
            ps_sq = lnps.tile([128, 512], F32, name="ps_sq", tag="lnsum")
            for nt in range(NT):
                sq = sqpool.tile([128, 512], F32R, name="sq", tag="sq")
                # square on ScalarE (idle in the tail) instead of VectorE
                nc.scalar.activation(out=sq, in_=src[:, nt, qs], func=AF.Square)
                nc.tensor.matmul(ps_sum[0:1, :], ones_sb[:, :], src[:, nt, qs],
                                 start=(nt == 0), stop=(nt == NT - 1))
                nc.tensor.matmul(ps_sq[0:1, :], ones_sb[:, :], sq[:, :],
                                 start=(nt == 0), stop=(nt == NT - 1))
            # mean, var, A = 1/sqrt(var+eps), B = -mean*A  (on [1, 512])
            nc.scalar.mul(out=m_sb[0:1, qs], in_=ps_sum[0:1, :], mul=1.0 / D)
            nc.scalar.mul(out=A[0:1, qs], in_=ps_sq[0:1, :], mul=1.0 / D)
            nc.vector.tensor_mul(out=Bt[0:1, qs], in0=m_sb[0:1, qs], in1=m_sb[0:1, qs])
            nc.vector.tensor_sub(out=A[0:1, qs], in0=A[0:1, qs], in1=Bt[0:1, qs])
            nc.scalar.activation(out=A[0:1, qs], in_=A[0:1, qs], func=AF.Sqrt,
                                 bias=eps_sb[0:1, 0:1], scale=1.0)
            nc.vector.reciprocal(out=A[0:1, qs], in_=A[0:1, qs])
            nc.vector.tensor_mul(out=Bt[0:1, qs], in0=m_sb[0:1, qs], in1=A[0:1, qs])
            nc.scalar.mul(out=Bt[0:1, qs], in_=Bt[0:1, qs], mul=-1.0)
    m_free()
    Abc, Abc_free = tc.tile([128, MQ], F32, name=f"Abc_{label}", side="right")
    Bbc, Bbc_free = tc.tile([128, MQ], F32, name=f"Bbc_{label}", side="right")
    bcast_row(nc, Abc[:, :], A[0:1, :], 128)
    bcast_row(nc, Bbc[:, :], Bt[0:1, :], 128)
    return (Abc, Bbc, (Bbc_free, Abc_free, B_free, A_free))


def apply_ln(nc, out_ap, y_ap, ab, g_col, b_col):
    Abc, Bbc, _ = ab
    nc.vector.tensor_mul(out=out_ap, in0=y_ap, in1=Abc)
    nc.vector.tensor_add(out=out_ap, in0=out_ap, in1=Bbc)
    nc.vector.tensor_scalar(
        out=out_ap, in0=out_ap, scalar1=g_col, scalar2=b_col,
        op0=mybir.AluOpType.mult, op1=mybir.AluOpType.add)


def layernorm(nc, tc, src, dst, g_sb, b_sb, ones_sb, eps_sb, label):
    ab = ln_stats(nc, tc, src=src, ones_sb=ones_sb, eps_sb=eps_sb, label=label)
    for nt in range(NT):
        apply_ln(nc, dst[:, nt, :], src[:, nt, :], ab,
                 g_sb[:, nt:nt + 1], b_sb[:, nt:nt + 1])
    for fr in ab[2]:
        fr()


_NC_CACHE = {}


def _get_nc():
    if "nc" not in _NC_CACHE:
        _NC_CACHE["nc"] = build_nc()
    return _NC_CACHE["nc"]


def kernel(x, wq, bq, wk, bk, wv, bv, wo, bo, wf, bf, g1, b1, g2, b2,
           _trace=False):
    nc = _get_nc()
    x = np.ascontiguousarray(np.asarray(x, dtype=np.float32))
    weights = {}
    for nm, v in (("wq", wq), ("wk", wk), ("wv", wv), ("wo", wo), ("wf", wf),
                  ("bq", bq), ("bk", bk), ("bv", bv), ("bo", bo), ("bf", bf),
                  ("g1", g1), ("b1", b1), ("g2", g2), ("b2", b2)):
        weights[nm] = np.ascontiguousarray(np.asarray(v, dtype=np.float32))
    in_maps = []
    for c in range(8):
        b, half = divmod(c, 2)
        xT = np.ascontiguousarray(x[b].T)                              # [D, S]
        xqTc = np.ascontiguousarray(x[b, half * MQ:(half + 1) * MQ].T)  # [D, MQ]
        in_maps.append({"xkvT": xT, "xqT": xqTc, **weights})
    res = run_bass_kernel_spmd(nc, in_maps, core_ids=list(range(8)), trace=_trace)
    out = np.empty((B, S, D), dtype=np.float32)
    for c in range(8):
        b, half = divmod(c, 2)
        out[b, half * MQ:(half + 1) * MQ, :] = res.results[c]["outT"].T
    if _trace:
        return out, res
    return out


# revision 20
# speedup vs baseline: 1.0340x; 1.0340x over previous
"""Trainium2 Bass kernel for a dense transformer encoder layer.

Shapes: B=4, S=2048, D=1024, H=16 heads x DH=64.
Sharding: sequence-parallel over 8 cores — core c handles batch c//2,
query rows (c%2)*1024..+1024; K/V computed for the full batch sequence
on both cores of a pair (duplicated, no collectives).

Layout strategy: all on-chip activations are kept transposed
[features -> partitions (tiles of 128), tokens -> free dim] so that every
matmul contraction lands on the partition axis without any on-chip
transposes.  The host pre-transposes x and post-transposes the output.

Precision: Q/O/FFN projections run fp32r (full PE rate at N=512, ~1.5e-4
matmul rel err); K/V projections and attention run bf16 with fp32 PSUM
accumulation.  Softmax skips max-subtraction (scores ~ N(0,1) after the
1/sqrt(dh) scale, folded into Exp's affine); the softmax denominator
comes from a ones-column appended to V, so attn @ [V|1] yields context
and sum-of-exponentials in one accumulation.  Context normalization is
deferred past the attention pipeline and the 1/sum_exp reciprocals are
batched into one multi-partition VectorE op.

Schedule: Q projection and the first half of V run up front; a
per-head-pair software pipeline then interleaves the K projection,
scores matmuls, Exp on ScalarE and context matmuls so TensorE and
ScalarE overlap continuously; the post-attention phases run qc-outer so
the two 512-token halves pipeline across engines.

SBUF is a two-sided stack allocator: long-lived tensors go on the left
stack, phase-transient ones on the right, freed in LIFO order per side.
"""

import numpy as np

import concourse.bass as bass
import concourse.mybir as mybir
import concourse.tile as tile
from concourse import bacc
from concourse.bass_utils import run_bass_kernel_spmd

F32 = mybir.dt.float32
F32R = mybir.dt.float32r
BF16 = mybir.dt.bfloat16
AF = mybir.ActivationFunctionType

B, S, D, H = 4, 2048, 1024, 16
DH = D // H          # 64
NT = D // 128        # 8 feature tiles
KTT = S // 128       # 16 key-token tiles
MQ = S // 2          # 1024 local query rows per core
QC = MQ // 512       # 2 query chunks of 512
EPS = 1e-6


def build_nc():
    nc = bacc.Bacc("TRN2", target_bir_lowering=False, debug=False, num_devices=8)

    xkvT = nc.declare_dram_parameter("xkvT", [D, S], F32, isOutput=False)
    xqT = nc.declare_dram_parameter("xqT", [D, MQ], F32, isOutput=False)
    w_par = {}
    b_par = {}
    for nm in ("wq", "wk", "wv", "wo", "wf"):
        w_par[nm] = nc.declare_dram_parameter(nm, [D, D], F32, isOutput=False)
    for nm in ("bq", "bk", "bv", "bo", "bf", "g1", "b1", "g2", "b2"):
        b_par[nm] = nc.declare_dram_parameter(nm, [D], F32, isOutput=False)
    outT = nc.declare_dram_parameter("outT", [D, MQ], F32, isOutput=True)

    with tile.TileContext(nc) as tc:
        keepalive = build_body(nc, tc, xkvT, xqT, w_par, b_par, outT)

    nc.compile()
    del keepalive
    return nc


def load_wcol(nc, pool, w, nt, dt=F32R, tag="wcol"):
    """weight column block w[:, nt*128:(nt+1)*128] -> SBUF [128, NT, 128]"""
    t = pool.tile([128, NT, 128], dt, name=f"wcol_{w.name}_{nt}", tag=tag)
    src = w[:, nt * 128:(nt + 1) * 128].rearrange("(t p) n -> p t n", p=128)
    nc.sync.dma_start(out=t, in_=src.bitcast(dt))
    return t


def load_wfull(nc, tc, w, dt=F32R):
    """full weight [D, D] -> SBUF [128, NT(kt), D(n)] on the left stack"""
    t, fr = tc.tile([128, NT, D], dt, name=f"wfull_{w.name}", side="left")
    nc.sync.dma_start(out=t,
                      in_=w[:, :].rearrange("(t p) n -> p t n", p=128).bitcast(dt))
    return t, fr


def bcast_row(nc, out_ap, row_ap, nparts):
    """replicate a [1, N] SBUF row across nparts partitions via DMA"""
    src = bass.AP(tensor=row_ap.tensor, offset=row_ap.offset,
                  ap=[list(row_ap.ap[0]), [0, nparts], list(row_ap.ap[-1])])
    nc.sync.dma_start(out=out_ap, in_=src)


def build_body(nc, tc, xkvT, xqT, w_par, b_par, outT):
    f32 = F32

    # ---------------- constants / biases (whole-kernel, left stack) ----------
    final_frees = []  # popped in reverse at the end (left-stack LIFO)
    bias_sb = {}
    for nm in ("bq", "bk", "bo", "bf", "g1", "b1", "g2", "b2"):
        t, fr = tc.tile([128, NT], f32, name=f"bias_{nm}", side="left")
        nc.sync.dma_start(out=t, in_=b_par[nm][:].rearrange("(t p) -> p t", p=128))
        bias_sb[nm] = t
        final_frees.append(fr)
    # bv broadcast across partitions [128, D] (free-dim bias for natural-layout V)
    bvb, bvb_fr = tc.tile([128, D], f32, name="bvb", side="left")
    final_frees.append(bvb_fr)
    bv_ap = b_par["bv"][:]
    bv_bc = bass.AP(tensor=bv_ap.tensor, offset=bv_ap.offset, ap=[[0, 128], [1, D]])
    nc.sync.dma_start(out=bvb, in_=bv_bc)
    ones_sb, ones_fr = tc.tile([128, 1], F32R, name="ones_sb", side="left")
    final_frees.append(ones_fr)
    ones_f32, ones_f32_fr = tc.tile([128, 1], f32, name="ones_f32", side="left")
    final_frees.append(ones_f32_fr)
    nc.vector.memset(ones_f32, 1.0)
    nc.scalar.copy(out=ones_sb, in_=ones_f32)
    eps_sb, eps_fr = tc.tile([1, 1], f32, name="eps_sb", side="left")
    final_frees.append(eps_fr)
    nc.vector.memset(eps_sb, EPS)

    # ---------------- x loads --------------------------------------------------
    # xq on the sync queue (feeds the very first matmuls); xkv streamed per-nt
    # on the gpsimd queue and converted to bf16 as tiles land.
    xkv16, xkv16_free = tc.tile([128, NT, S], BF16, name="xkv16", side="left")
    QT, QT_free = tc.tile([128, NT, MQ], BF16, name="QT", side="left")

    xkv_f32, xkv_f32_free = tc.tile([128, NT, S], f32, name="xkv_f32", side="right")
    xq_sb, xq_free = tc.tile([128, NT, MQ], F32R, name="xq_sb", side="right")
    nc.sync.dma_start(
        out=xq_sb,
        in_=xqT[:, :].rearrange("(t p) m -> p t m", p=128).bitcast(F32R))
    xkv_view = xkvT[:, :].rearrange("(t p) m -> p t m", p=128)
    for nt in range(NT):
        nc.gpsimd.dma_start(out=xkv_f32[:, nt, :], in_=xkv_view[:, nt, :])
        nc.vector.tensor_copy(out=xkv16[:, nt, :], in_=xkv_f32[:, nt, :])

    # ======================= Q projection (fp32r) ============================
    with tc.tile_pool(name="qps", bufs=4, space="PSUM") as qps:
        with tc.tile_pool(name="wqpool", bufs=2, side="left") as wqpool:
            for nt in range(NT):
                wc = load_wcol(nc, wqpool, w_par["wq"], nt)
                for qc in range(QC):
                    ps = qps.tile([128, 512], f32, name="ps_q", tag="p1")
                    for kt in range(NT):
                        nc.tensor.matmul(
                            ps, wc[:, kt, :],
                            xq_sb[:, kt, qc * 512:(qc + 1) * 512],
                            start=(kt == 0), stop=(kt == NT - 1))
                    nc.vector.tensor_scalar(
                        out=QT[:, nt, qc * 512:(qc + 1) * 512], in0=ps,
                        scalar1=bias_sb["bq"][:, nt:nt + 1], scalar2=None,
                        op0=mybir.AluOpType.add)
        xq_free()
    xkv_f32_free()

    # ======================= attention ======================================
    # V projection (bf16) half dc fills VA[:, :, dc*8:(dc+1)*8, :]; the
    # per-head-pair pipeline starts after the first half.
    VA, VA_free = tc.tile([128, KTT, H, DH + 1], BF16, name="VA", side="left")
    ctxT, ctxT_free = tc.tile([128, NT, MQ], F32R, name="ctxT", side="right")
    se_all, se_free = tc.tile([16, MQ], f32, name="se_all", side="right")

    with tc.tile_pool(name="wvpool", bufs=1, side="left") as wvpool, \
         tc.tile_pool(name="wkpool", bufs=2, side="left") as wkpool, \
         tc.tile_pool(name="ktpool", bufs=2, side="left") as ktpool, \
         tc.tile_pool(name="exppool", bufs=4, side="left") as exppool, \
         tc.tile_pool(name="sepool", bufs=4, side="left") as sepool, \
         tc.tile_pool(name="vps", bufs=2, space="PSUM") as vps, \
         tc.tile_pool(name="sps", bufs=2, space="PSUM") as sps, \
         tc.tile_pool(name="cps", bufs=2, space="PSUM") as cps:

        def v_proj_half(dc):
            wv_f32 = wvpool.tile([128, NT, 512], f32,
                                 name=f"wv_f32_{dc}", tag="wvf")
            nc.sync.dma_start(
                out=wv_f32,
                in_=w_par["wv"][:, dc * 512:(dc + 1) * 512].rearrange(
                    "(t p) n -> p t n", p=128))
            wv16 = wvpool.tile([128, NT, 512], BF16, name=f"wv16_{dc}", tag="wv16")
            nc.vector.tensor_copy(out=wv16, in_=wv_f32)
            for tt in range(KTT):
                ps = vps.tile([128, 512], f32, name="ps_v", tag="pv")
                for kt in range(NT):
                    nc.tensor.matmul(
                        ps, xkv16[:, kt, tt * 128:(tt + 1) * 128],
                        wv16[:, kt, :],
                        start=(kt == 0), stop=(kt == NT - 1))
                nc.vector.tensor_add(
                    out=VA[:, tt, dc * 8:(dc + 1) * 8, 0:DH],
                    in0=ps.rearrange("p (h d) -> p h d", h=8),
                    in1=bvb[:, dc * 512:(dc + 1) * 512].rearrange(
                        "p (h d) -> p h d", h=8))
                nc.vector.memset(VA[:, tt, dc * 8:(dc + 1) * 8, DH:DH + 1], 1.0)

        def attention_hp(hp):
            h0, h1 = 2 * hp, 2 * hp + 1
            # K projection for feature tile hp over the full sequence
            wk_f32 = wkpool.tile([128, NT, 128], f32, name="wk_f32", tag="wkf")
            nc.sync.dma_start(
                out=wk_f32,
                in_=w_par["wk"][:, hp * 128:(hp + 1) * 128].rearrange(
                    "(t p) n -> p t n", p=128))
            wk16 = wkpool.tile([128, NT, 128], BF16, name="wk16", tag="wk16")
            nc.vector.tensor_copy(out=wk16, in_=wk_f32)
            KTh = ktpool.tile([128, S], BF16, name="KTh", tag="kth")
            for mc in range(S // 512):
                ps = vps.tile([128, 512], f32, name="ps_k", tag="pv")
                for kt in range(NT):
                    nc.tensor.matmul(
                        ps, wk16[:, kt, :],
                        xkv16[:, kt, mc * 512:(mc + 1) * 512],
                        start=(kt == 0), stop=(kt == NT - 1))
                nc.vector.tensor_scalar(
                    out=KTh[:, mc * 512:(mc + 1) * 512], in0=ps,
                    scalar1=bias_sb["bk"][:, hp:hp + 1], scalar2=None,
                    op0=mybir.AluOpType.add)

            # scores -> exp -> ctx, one-group software pipeline
            for qc in range(QC):
                qs = slice(qc * 512, (qc + 1) * 512)
                pc0 = cps.tile([128, 512], f32, name="pc0", tag="pc")
                pc1 = cps.tile([128, 512], f32, name="pc1", tag="pc")
                e_by_half = {}

                def emit_ctx(half, g):
                    e0g, e1g = e_by_half[half]
                    for j in range(2):
                        kt = half * 8 + g * 2 + j
                        nc.tensor.matmul(
                            pc0[0:DH + 1, :], VA[:, kt, h0, :],
                            e0g[:, g * 2 + j, :],
                            start=(kt == 0), stop=(kt == KTT - 1))
                        nc.tensor.matmul(
                            pc1[0:DH + 1, :], VA[:, kt, h1, :],
                            e1g[:, g * 2 + j, :],
                            start=(kt == 0), stop=(kt == KTT - 1))

                groups = [(half, g) for half in range(2) for g in range(4)]
                for idx, (half, g) in enumerate(groups):
                    if g == 0:
                        e_by_half[half] = (
                            exppool.tile([128, 8, 512], BF16, name="e0", tag="exp"),
                            exppool.tile([128, 8, 512], BF16, name="e1", tag="exp"))
                    ps0 = sps.tile([128, 2, 512], f32, name="ps0", tag="sc")
                    ps1 = sps.tile([128, 2, 512], f32, name="ps1", tag="sc")
                    for j in range(2):
                        kt = half * 8 + g * 2 + j
                        ks = slice(kt * 128, (kt + 1) * 128)
                        nc.tensor.matmul(
                            ps0[:, j, :], KTh[0:64, ks], QT[0:64, hp, qs],
                            start=True, stop=True, tile_position=(0, 0))
                        nc.tensor.matmul(
                            ps1[:, j, :], KTh[64:128, ks], QT[64:128, hp, qs],
                            start=True, stop=True, tile_position=(64, 0))
                    e0g, e1g = e_by_half[half]
                    nc.scalar.activation(
                        out=e0g[:, g * 2:(g + 1) * 2, :], in_=ps0,
                        func=AF.Exp, scale=1.0 / np.sqrt(DH))
                    nc.scalar.activation(
                        out=e1g[:, g * 2:(g + 1) * 2, :], in_=ps1,
                        func=AF.Exp, scale=1.0 / np.sqrt(DH))
                    if idx > 0:
                        emit_ctx(*groups[idx - 1])
                emit_ctx(*groups[-1])

                # stash unnormalized ctx + sum_exp rows; normalize later.
                # (DMA cannot read PSUM: bounce the rows through partition 0.)
                t0 = sepool.tile([1, 512], f32, name="t0", tag="set")
                t1 = sepool.tile([1, 512], f32, name="t1", tag="set")
                nc.vector.tensor_copy(out=t0, in_=pc0[DH:DH + 1, :])
                nc.vector.tensor_copy(out=t1, in_=pc1[DH:DH + 1, :])
                nc.sync.dma_start(out=se_all[h0:h0 + 1, qs], in_=t0)
                nc.sync.dma_start(out=se_all[h1:h1 + 1, qs], in_=t1)
                nc.vector.tensor_copy(out=ctxT[0:64, hp, qs], in_=pc0[0:DH, :])
                nc.vector.tensor_copy(out=ctxT[64:128, hp, qs], in_=pc1[0:DH, :])

        v_proj_half(0)
        for hp in range(4):
            attention_hp(hp)
        v_proj_half(1)
        for hp in range(4, 8):
            attention_hp(hp)

    VA_free()
    QT_free()
    xkv16_free()

    # ---- deferred ctx normalization: one batched reciprocal, then per-hp mul
    se_inv, se_inv_free = tc.tile([16, MQ], f32, name="se_inv", side="right")
    nc.vector.reciprocal(out=se_inv, in_=se_all)
    with tc.tile_pool(name="bcpool", bufs=2, side="left") as bcpool:
        for hp in range(H // 2):
            h0, h1 = 2 * hp, 2 * hp + 1
            bc = bcpool.tile([128, MQ], f32, name="bc", tag="bc")
            bcast_row(nc, bc[0:64, :], se_inv[h0:h0 + 1, :], 64)
            bcast_row(nc, bc[64:128, :], se_inv[h1:h1 + 1, :], 64)
            nc.vector.tensor_mul(out=ctxT[:, hp, :], in0=ctxT[:, hp, :], in1=bc)
    se_inv_free()
    se_free()

    # ======================= P3..P6: O-proj, LN1, FFN, LN2 ===================
    # qc-outer so the two 512-token halves pipeline across PE/ACT/DVE.
    y1, y1_free = tc.tile([128, NT, MQ], F32R, name="y1", side="left")
    wo_sb, wo_fr = load_wfull(nc, tc, w_par["wo"])
    xq2, xq2_free = tc.tile([128, NT, MQ], f32, name="xq2", side="right")
    nc.sync.dma_start(out=xq2, in_=xqT[:, :].rearrange("(t p) m -> p t m", p=128))

    with tc.tile_pool(name="p3ps", bufs=4, space="PSUM") as p3ps:
        for qc in range(QC):
            qs = slice(qc * 512, (qc + 1) * 512)
            for nt in range(NT):
                ps = p3ps.tile([128, 512], f32, name="ps_o", tag="p3")
                for kt in range(NT):
                    nc.tensor.matmul(
                        ps, wo_sb[:, kt, nt * 128:(nt + 1) * 128], ctxT[:, kt, qs],
                        start=(kt == 0), stop=(kt == NT - 1))
                nc.vector.tensor_scalar(
                    out=y1[:, nt, qs], in0=ps,
                    scalar1=bias_sb["bo"][:, nt:nt + 1], scalar2=None,
                    op0=mybir.AluOpType.add)
                nc.vector.tensor_add(out=y1[:, nt, qs], in0=y1[:, nt, qs],
                                     in1=xq2[:, nt, qs])
    xq2_free()
    ctxT_free()
    wo_fr()

    # LN1 -> n1
    n1, n1_free = tc.tile([128, NT, MQ], F32R, name="n1", side="right")
    layernorm(nc, tc, src=y1, dst=n1, g_sb=bias_sb["g1"], b_sb=bias_sb["b1"],
              ones_sb=ones_sb, eps_sb=eps_sb, label="ln1")
    y1_free()

    # FFN -> fnn, then y2 = fnn + n1 (overwrite fnn)
    wf_sb, wf_fr = load_wfull(nc, tc, w_par["wf"])
    fnn, fnn_free = tc.tile([128, NT, MQ], F32R, name="fnn", side="left")
    with tc.tile_pool(name="p5ps", bufs=4, space="PSUM") as p5ps:
        for qc in range(QC):
            qs = slice(qc * 512, (qc + 1) * 512)
            for nt in range(NT):
                ps = p5ps.tile([128, 512], f32, name="ps_f", tag="p5")
                for kt in range(NT):
                    nc.tensor.matmul(
                        ps, wf_sb[:, kt, nt * 128:(nt + 1) * 128], n1[:, kt, qs],
                        start=(kt == 0), stop=(kt == NT - 1))
                nc.scalar.activation(
                    out=fnn[:, nt, qs], in_=ps, func=AF.Relu,
                    bias=bias_sb["bf"][:, nt:nt + 1], scale=1.0)
            for nt in range(NT):
                nc.vector.tensor_add(out=fnn[:, nt, qs], in0=fnn[:, nt, qs],
                                     in1=n1[:, nt, qs])
    n1_free()

    # LN2 -> out
    with tc.tile_pool(name="outpool", bufs=3, side="left") as outpool:
        ab = ln_stats(nc, tc, src=fnn, ones_sb=ones_sb, eps_sb=eps_sb, label="ln2")
        for qc in range(QC):
            qs = slice(qc * 512, (qc + 1) * 512)
            for nt in range(NT):
                n2t = outpool.tile([128, 512], f32, name="n2t", tag="n2")
                apply_ln(nc, n2t, fnn[:, nt, qs], ab, qs,
                         bias_sb["g2"][:, nt:nt + 1], bias_sb["b2"][:, nt:nt + 1])
                nc.sync.dma_start(out=outT[nt * 128:(nt + 1) * 128, qs], in_=n2t)
        for fr in ab[2]:
            fr()
    fnn_free()
    wf_fr()

    for fr in reversed(final_frees):
        fr()
    return []


def ln_stats(nc, tc, src, ones_sb, eps_sb, label):
    """column sums of src and src^2 over all D partitions via ones-matmuls;
    returns (Abc, Bbc, frees): broadcast tiles with A=rstd, B=-mean*rstd."""
    A, A_free = tc.tile([1, MQ], F32, name=f"A_{label}", side="right")
    Bt, B_free = tc.tile([1, MQ], F32, name=f"B_{label}", side="right")
    m_sb, m_free = tc.tile([1, MQ], F32, name=f"m_{label}", side="right")
    with tc.tile_pool(name=f"sq_{label}", bufs=2, side="left") as sqpool, \
         tc.tile_pool(name=f"lnps_{label}", bufs=2, space="PSUM") as lnps:
        for qc in range(QC):
            qs = slice(qc * 512, (qc + 1) * 512)
            ps_sum = lnps.tile([128, 512], F32, name="ps_sum", tag="lnsum")
            ps_sq = lnps.tile([128, 512], F32, name="ps_sq", tag="lnsum")
            for nt in range(NT):
                sq = sqpool.tile([128, 512], F32R, name="sq", tag="sq")
                # square on ScalarE (idle in the tail) instead of VectorE
                nc.scalar.activation(out=sq, in_=src[:, nt, qs], func=AF.Square)
                nc.tensor.matmul(ps_sum[0:1, :], ones_sb[:, :], src[:, nt, qs],
                                 start=(nt == 0), stop=(nt == NT - 1))
                nc.tensor.matmul(ps_sq[0:1, :], ones_sb[:, :], sq[:, :],
                                 start=(nt == 0), stop=(nt == NT - 1))
            # mean, var, A = rsqrt(var+eps), B = -mean*A  (on [1, 512])
            nc.scalar.mul(out=m_sb[0:1, qs], in_=ps_sum[0:1, :], mul=1.0 / D)
            nc.scalar.mul(out=A[0:1, qs], in_=ps_sq[0:1, :], mul=1.0 / D)
            nc.vector.tensor_mul(out=Bt[0:1, qs], in0=m_sb[0:1, qs], in1=m_sb[0:1, qs])
            nc.vector.tensor_sub(out=A[0:1, qs], in0=A[0:1, qs], in1=Bt[0:1, qs])
            nc.scalar.activation(out=A[0:1, qs], in_=A[0:1, qs], func=AF.Sqrt,
                                 bias=eps_sb[0:1, 0:1], scale=1.0)
            nc.vector.reciprocal(out=A[0:1, qs], in_=A[0:1, qs])
            nc.vector.tensor_mul(out=Bt[0:1, qs], in0=m_sb[0:1, qs], in1=A[0:1, qs])
            nc.scalar.mul(out=Bt[0:1, qs], in_=Bt[0:1, qs], mul=-1.0)
    m_free()
    Abc, Abc_free = tc.tile([128, MQ], F32, name=f"Abc_{label}", side="right")
    Bbc, Bbc_free = tc.tile([128, MQ], F32, name=f"Bbc_{label}", side="right")
    bcast_row(nc, Abc[:, :], A[0:1, :], 128)
    bcast_row(nc, Bbc[:, :], Bt[0:1, :], 128)
    return (Abc, Bbc, (Bbc_free, Abc_free, B_free, A_free))


def apply_ln(nc, out_ap, y_ap, ab, qs, g_col, b_col):
    Abc, Bbc, _ = ab
    nc.vector.tensor_mul(out=out_ap, in0=y_ap, in1=Abc[:, qs])
    nc.vector.tensor_add(out=out_ap, in0=out_ap, in1=Bbc[:, qs])
    nc.vector.tensor_scalar(
        out=out_ap, in0=out_ap, scalar1=g_col, scalar2=b_col,
        op0=mybir.AluOpType.mult, op1=mybir.AluOpType.add)


def layernorm(nc, tc, src, dst, g_sb, b_sb, ones_sb, eps_sb, label):
    ab = ln_stats(nc, tc, src=src, ones_sb=ones_sb, eps_sb=eps_sb, label=label)
    for qc in range(QC):
        qs = slice(qc * 512, (qc + 1) * 512)
        for nt in range(NT):
            apply_ln(nc, dst[:, nt, qs], src[:, nt, qs], ab, qs,
                     g_sb[:, nt:nt + 1], b_sb[:, nt:nt + 1])
    for fr in ab[2]:
        fr()


_NC_CACHE = {}


def _get_nc():
    if "nc" not in _NC_CACHE:
        _NC_CACHE["nc"] = build_nc()
    return _NC_CACHE["nc"]


def kernel(x, wq, bq, wk, bk, wv, bv, wo, bo, wf, bf, g1, b1, g2, b2,
           _trace=False):
    nc = _get_nc()
    x = np.ascontiguousarray(np.asarray(x, dtype=np.float32))
    weights = {}
    for nm, v in (("wq", wq), ("wk", wk), ("wv", wv), ("wo", wo), ("wf", wf),
                  ("bq", bq), ("bk", bk), ("bv", bv), ("bo", bo), ("bf", bf),
                  ("g1", g1), ("b1", b1), ("g2", g2), ("b2", b2)):
        weights[nm] = np.ascontiguousarray(np.asarray(v, dtype=np.float32))
    in_maps = []
    for c in range(8):
        b, half = divmod(c, 2)
        xT = np.ascontiguousarray(x[b].T)                              # [D, S]
        xqTc = np.ascontiguousarray(x[b, half * MQ:(half + 1) * MQ].T)  # [D, MQ]
        in_maps.append({"xkvT": xT, "xqT": xqTc, **weights})
    res = run_bass_kernel_spmd(nc, in_maps, core_ids=list(range(8)), trace=_trace)
    out = np.empty((B, S, D), dtype=np.float32)
    for c in range(8):
        b, half = divmod(c, 2)
        out[b, half * MQ:(half + 1) * MQ, :] = res.results[c]["outT"].T
    if _trace:
        return out, res
    return out


# revision 24
# speedup vs baseline: 1.1964x; 1.1571x over previous
"""Trainium2 Bass kernel for a dense transformer encoder layer.

Shapes: B=4, S=2048, D=1024, H=16 heads x DH=64.
Sharding: sequence-parallel over 8 cores — core c handles batch c//2,
query rows (c%2)*1024..+1024; K/V computed for the full batch sequence
on both cores of a pair (duplicated, no collectives).

Layout strategy: all on-chip activations are kept transposed
[features -> partitions (tiles of 128), tokens -> free dim] so that every
matmul contraction lands on the partition axis without any on-chip
transposes.  The host pre-transposes x and post-transposes the output.

Precision: Q/O/FFN projections run fp32r (full PE rate at N=512, ~1.5e-4
matmul rel err); K/V projections and attention run bf16 with fp32 PSUM
accumulation.  Softmax skips max-subtraction (scores ~ N(0,1) after the
1/sqrt(dh) scale, folded into Exp's affine); the softmax denominator
comes from a ones-column appended to V, so attn @ [V|1] yields context
and sum-of-exponentials in one accumulation.  Context normalization is
deferred past the attention pipeline and the 1/sum_exp reciprocals are
batched into one multi-partition VectorE op.

Schedule: Q projection and the first half of V run up front; a
per-head-pair software pipeline then interleaves the K projection,
scores matmuls, Exp on ScalarE and context matmuls so TensorE and
ScalarE overlap continuously; LayerNorm statistics are broadcast across
partitions with ones-matmuls into PSUM (no DMA on the critical path).

SBUF is a two-sided stack allocator: long-lived tensors go on the left
stack, phase-transient ones on the right, freed in LIFO order per side.
"""

import numpy as np

import concourse.bass as bass
import concourse.mybir as mybir
import concourse.tile as tile
from concourse import bacc
from concourse.bass_utils import run_bass_kernel_spmd

F32 = mybir.dt.float32
F32R = mybir.dt.float32r
BF16 = mybir.dt.bfloat16
AF = mybir.ActivationFunctionType

B, S, D, H = 4, 2048, 1024, 16
DH = D // H          # 64
NT = D // 128        # 8 feature tiles
KTT = S // 128       # 16 key-token tiles
MQ = S // 2          # 1024 local query rows per core
QC = MQ // 512       # 2 query chunks of 512
EPS = 1e-6


def build_nc():
    nc = bacc.Bacc("TRN2", target_bir_lowering=False, debug=False, num_devices=8)

    xkvT = nc.declare_dram_parameter("xkvT", [D, S], F32, isOutput=False)
    xqT = nc.declare_dram_parameter("xqT", [D, MQ], F32, isOutput=False)
    w_par = {}
    b_par = {}
    for nm in ("wq", "wk", "wv", "wo", "wf"):
        w_par[nm] = nc.declare_dram_parameter(nm, [D, D], F32, isOutput=False)
    for nm in ("bq", "bk", "bv", "bo", "bf", "g1", "b1", "g2", "b2"):
        b_par[nm] = nc.declare_dram_parameter(nm, [D], F32, isOutput=False)
    outT = nc.declare_dram_parameter("outT", [D, MQ], F32, isOutput=True)

    with tile.TileContext(nc) as tc:
        keepalive = build_body(nc, tc, xkvT, xqT, w_par, b_par, outT)

    nc.compile()
    del keepalive
    return nc


def load_wcol(nc, pool, w, nt, dt=F32R, tag="wcol", eng=None):
    """weight column block w[:, nt*128:(nt+1)*128] -> SBUF [128, NT, 128]"""
    t = pool.tile([128, NT, 128], dt, name=f"wcol_{w.name}_{nt}", tag=tag)
    src = w[:, nt * 128:(nt + 1) * 128].rearrange("(t p) n -> p t n", p=128)
    (eng or nc.sync).dma_start(out=t, in_=src.bitcast(dt))
    return t


def bcast_row(nc, out_ap, row_ap, nparts, eng=None):
    """replicate a [1, N] SBUF row across nparts partitions via DMA"""
    src = bass.AP(tensor=row_ap.tensor, offset=row_ap.offset,
                  ap=[list(row_ap.ap[0]), [0, nparts], list(row_ap.ap[-1])])
    (eng or nc.sync).dma_start(out=out_ap, in_=src)


def build_body(nc, tc, xkvT, xqT, w_par, b_par, outT):
    f32 = F32

    # ---------------- allocations: constants bottom of left stack -----------
    final_frees = []  # popped in reverse at the end (left-stack LIFO)
    bias_sb = {}
    for nm in ("bq", "bk", "bo", "bf", "g1", "b1", "g2", "b2"):
        t, fr = tc.tile([128, NT], f32, name=f"bias_{nm}", side="left")
        bias_sb[nm] = t
        final_frees.append(fr)
    bvb, bvb_fr = tc.tile([128, D], f32, name="bvb", side="left")
    final_frees.append(bvb_fr)
    ones_sb, ones_fr = tc.tile([128, 1], F32R, name="ones_sb", side="left")
    final_frees.append(ones_fr)
    onesr_sb, onesr_fr = tc.tile([1, 128], f32, name="onesr_sb", side="left")
    final_frees.append(onesr_fr)
    ones_f32, ones_f32_fr = tc.tile([128, 128], f32, name="ones_f32", side="left")
    final_frees.append(ones_f32_fr)
    eps_sb, eps_fr = tc.tile([1, 1], f32, name="eps_sb", side="left")
    final_frees.append(eps_fr)
    xkv16, xkv16_free = tc.tile([128, NT, S], BF16, name="xkv16", side="left")
    QT, QT_free = tc.tile([128, NT, MQ], BF16, name="QT", side="left")
    xkv_f32, xkv_f32_free = tc.tile([128, NT, S], f32, name="xkv_f32", side="right")
    xq_sb, xq_free = tc.tile([128, NT, MQ], F32R, name="xq_sb", side="right")

    # ---------------- x loads first (feed the very first matmuls) ------------
    xq_view = xqT[:, :].rearrange("(t p) m -> p t m", p=128).bitcast(F32R)
    for nt in range(NT):
        nc.sync.dma_start(out=xq_sb[:, nt, :], in_=xq_view[:, nt, :])
    xkv_view = xkvT[:, :].rearrange("(t p) m -> p t m", p=128)
    for nt in range(NT):
        nc.gpsimd.dma_start(out=xkv_f32[:, nt, :], in_=xkv_view[:, nt, :])
        nc.vector.tensor_copy(out=xkv16[:, nt, :], in_=xkv_f32[:, nt, :])

    # ---------------- constants / biases ------------------------------------
    for nm in ("bq", "bk", "bo", "bf", "g1", "b1", "g2", "b2"):
        nc.scalar.dma_start(out=bias_sb[nm],
                            in_=b_par[nm][:].rearrange("(t p) -> p t", p=128))
    bv_ap = b_par["bv"][:]
    bv_bc = bass.AP(tensor=bv_ap.tensor, offset=bv_ap.offset, ap=[[0, 128], [1, D]])
    nc.scalar.dma_start(out=bvb, in_=bv_bc)
    nc.vector.memset(ones_f32, 1.0)
    nc.scalar.copy(out=ones_sb, in_=ones_f32[:, 0:1])
    nc.scalar.copy(out=onesr_sb, in_=ones_f32[0:1, :])
    nc.vector.memset(eps_sb, EPS)

    # ======================= Q projection (fp32r) ============================
    with tc.tile_pool(name="qps", bufs=4, space="PSUM") as qps:
        with tc.tile_pool(name="wqpool", bufs=2, side="left") as wqpool:
            for nt in range(NT):
                wc = load_wcol(nc, wqpool, w_par["wq"], nt)
                for qc in range(QC):
                    ps = qps.tile([128, 512], f32, name="ps_q", tag="p1")
                    for kt in range(NT):
                        nc.tensor.matmul(
                            ps, wc[:, kt, :],
                            xq_sb[:, kt, qc * 512:(qc + 1) * 512],
                            start=(kt == 0), stop=(kt == NT - 1))
                    nc.vector.tensor_scalar(
                        out=QT[:, nt, qc * 512:(qc + 1) * 512], in0=ps,
                        scalar1=bias_sb["bq"][:, nt:nt + 1], scalar2=None,
                        op0=mybir.AluOpType.add)
        xq_free()
    xkv_f32_free()

    # ======================= attention ======================================
    # V projection (bf16) half dc fills VA[:, :, dc*8:(dc+1)*8, :]; the
    # per-head-pair pipeline starts after the first half.
    VA, VA_free = tc.tile([128, KTT, H, DH + 1], BF16, name="VA", side="left")
    ctxT, ctxT_free = tc.tile([128, NT, MQ], F32R, name="ctxT", side="right")
    se_all, se_free = tc.tile([16, MQ], f32, name="se_all", side="right")

    with tc.tile_pool(name="wvpool", bufs=1, side="left") as wvpool, \
         tc.tile_pool(name="wkpool", bufs=2, side="left") as wkpool, \
         tc.tile_pool(name="ktpool", bufs=2, side="left") as ktpool, \
         tc.tile_pool(name="exppool", bufs=4, side="left") as exppool, \
         tc.tile_pool(name="sepool", bufs=4, side="left") as sepool, \
         tc.tile_pool(name="sps", bufs=3, space="PSUM") as sps, \
         tc.tile_pool(name="cps", bufs=2, space="PSUM") as cps:

        def v_proj_half(dc):
            wv_f32 = wvpool.tile([128, NT, 512], f32,
                                 name=f"wv_f32_{dc}", tag="wvf")
            nc.sync.dma_start(
                out=wv_f32,
                in_=w_par["wv"][:, dc * 512:(dc + 1) * 512].rearrange(
                    "(t p) n -> p t n", p=128))
            wv16 = wvpool.tile([128, NT, 512], BF16, name=f"wv16_{dc}", tag="wv16")
            nc.vector.tensor_copy(out=wv16, in_=wv_f32)
            for ttp in range(KTT // 2):
                ps = sps.tile([128, 2, 512], f32, name="ps_v", tag="sc")
                for j in range(2):
                    tt = 2 * ttp + j
                    for kt in range(NT):
                        nc.tensor.matmul(
                            ps[:, j, :], xkv16[:, kt, tt * 128:(tt + 1) * 128],
                            wv16[:, kt, :],
                            start=(kt == 0), stop=(kt == NT - 1))
                for j in range(2):
                    tt = 2 * ttp + j
                    nc.vector.tensor_add(
                        out=VA[:, tt, dc * 8:(dc + 1) * 8, 0:DH],
                        in0=ps[:, j, :].rearrange("p (h d) -> p h d", h=8),
                        in1=bvb[:, dc * 512:(dc + 1) * 512].rearrange(
                            "p (h d) -> p h d", h=8))
                    nc.vector.memset(
                        VA[:, tt, dc * 8:(dc + 1) * 8, DH:DH + 1], 1.0)

        def attention_hp(hp):
            h0, h1 = 2 * hp, 2 * hp + 1
            # K projection for feature tile hp over the full sequence
            wk_f32 = wkpool.tile([128, NT, 128], f32, name="wk_f32", tag="wkf")
            nc.sync.dma_start(
                out=wk_f32,
                in_=w_par["wk"][:, hp * 128:(hp + 1) * 128].rearrange(
                    "(t p) n -> p t n", p=128))
            wk16 = wkpool.tile([128, NT, 128], BF16, name="wk16", tag="wk16")
            nc.vector.tensor_copy(out=wk16, in_=wk_f32)
            KTh = ktpool.tile([128, S], BF16, name="KTh", tag="kth")
            for mcp in range(S // 1024):
                ps = sps.tile([128, 2, 512], f32, name="ps_k", tag="sc")
                for j in range(2):
                    mc = 2 * mcp + j
                    for kt in range(NT):
                        nc.tensor.matmul(
                            ps[:, j, :], wk16[:, kt, :],
                            xkv16[:, kt, mc * 512:(mc + 1) * 512],
                            start=(kt == 0), stop=(kt == NT - 1))
                nc.vector.tensor_scalar(
                    out=KTh[:, mcp * 1024:(mcp + 1) * 1024],
                    in0=ps.rearrange("p a b -> p (a b)"),
                    scalar1=bias_sb["bk"][:, hp:hp + 1], scalar2=None,
                    op0=mybir.AluOpType.add)

            # scores -> exp -> ctx, one-group software pipeline
            for qc in range(QC):
                qs = slice(qc * 512, (qc + 1) * 512)
                pc0 = cps.tile([128, 512], f32, name="pc0", tag="pc")
                pc1 = cps.tile([128, 512], f32, name="pc1", tag="pc")
                e_by_half = {}

                def emit_ctx(half, g):
                    e0g, e1g = e_by_half[half]
                    for j in range(2):
                        kt = half * 8 + g * 2 + j
                        nc.tensor.matmul(
                            pc0[0:DH + 1, :], VA[:, kt, h0, :],
                            e0g[:, g * 2 + j, :],
                            start=(kt == 0), stop=(kt == KTT - 1))
                        nc.tensor.matmul(
                            pc1[0:DH + 1, :], VA[:, kt, h1, :],
                            e1g[:, g * 2 + j, :],
                            start=(kt == 0), stop=(kt == KTT - 1))

                groups = [(half, g) for half in range(2) for g in range(4)]
                for idx, (half, g) in enumerate(groups):
                    if g == 0:
                        e_by_half[half] = (
                            exppool.tile([128, 8, 512], BF16, name="e0", tag="exp"),
                            exppool.tile([128, 8, 512], BF16, name="e1", tag="exp"))
                    ps0 = sps.tile([128, 2, 512], f32, name="ps0", tag="sc")
                    ps1 = sps.tile([128, 2, 512], f32, name="ps1", tag="sc")
                    for j in range(2):
                        kt = half * 8 + g * 2 + j
                        ks = slice(kt * 128, (kt + 1) * 128)
                        nc.tensor.matmul(
                            ps0[:, j, :], KTh[0:64, ks], QT[0:64, hp, qs],
                            start=True, stop=True, tile_position=(0, 0))
                        nc.tensor.matmul(
                            ps1[:, j, :], KTh[64:128, ks], QT[64:128, hp, qs],
                            start=True, stop=True, tile_position=(64, 0))
                    e0g, e1g = e_by_half[half]
                    nc.scalar.activation(
                        out=e0g[:, g * 2:(g + 1) * 2, :], in_=ps0,
                        func=AF.Exp, scale=1.0 / np.sqrt(DH))
                    nc.scalar.activation(
                        out=e1g[:, g * 2:(g + 1) * 2, :], in_=ps1,
                        func=AF.Exp, scale=1.0 / np.sqrt(DH))
                    if idx > 0:
                        emit_ctx(*groups[idx - 1])
                emit_ctx(*groups[-1])

                # stash unnormalized ctx + sum_exp rows; normalize later.
                # (DMA cannot read PSUM: bounce the rows through partition 0.)
                t0 = sepool.tile([1, 512], f32, name="t0", tag="set")
                t1 = sepool.tile([1, 512], f32, name="t1", tag="set")
                nc.vector.tensor_copy(out=t0, in_=pc0[DH:DH + 1, :])
                nc.vector.tensor_copy(out=t1, in_=pc1[DH:DH + 1, :])
                nc.gpsimd.dma_start(out=se_all[h0:h0 + 1, qs], in_=t0)
                nc.gpsimd.dma_start(out=se_all[h1:h1 + 1, qs], in_=t1)
                nc.vector.tensor_copy(out=ctxT[0:64, hp, qs], in_=pc0[0:DH, :])
                nc.vector.tensor_copy(out=ctxT[64:128, hp, qs], in_=pc1[0:DH, :])

        v_proj_half(0)
        for hp in range(4):
            attention_hp(hp)
        v_proj_half(1)
        for hp in range(4, 8):
            attention_hp(hp)

    VA_free()
    QT_free()
    xkv16_free()

    # ---- deferred ctx normalization: one batched reciprocal, then per-hp mul
    dma_engs = [nc.sync, nc.gpsimd, nc.scalar]
    se_inv, se_inv_free = tc.tile([16, MQ], f32, name="se_inv", side="right")
    nc.vector.reciprocal(out=se_inv, in_=se_all)
    with tc.tile_pool(name="bcpool", bufs=3, side="left") as bcpool:
        for hp in range(H // 2):
            h0, h1 = 2 * hp, 2 * hp + 1
            bc = bcpool.tile([128, MQ], f32, name="bc", tag="bc")
            bcast_row(nc, bc[0:64, :], se_inv[h0:h0 + 1, :], 64,
                      eng=dma_engs[hp % 3])
            bcast_row(nc, bc[64:128, :], se_inv[h1:h1 + 1, :], 64,
                      eng=dma_engs[(hp + 1) % 3])
            nc.vector.tensor_mul(out=ctxT[:, hp, :], in0=ctxT[:, hp, :], in1=bc)
    se_inv_free()
    se_free()

    # ======================= P3: output projection + residual ================
    y1, y1_free = tc.tile([128, NT, MQ], F32R, name="y1", side="left")
    with tc.tile_pool(name="wpool3", bufs=3, side="left") as wpool3, \
         tc.tile_pool(name="xq2pool", bufs=3, side="left") as xq2pool, \
         tc.tile_pool(name="p3ps", bufs=4, space="PSUM") as p3ps:
        xq2_view = xqT[:, :].rearrange("(t p) m -> p t m", p=128)
        for nt in range(NT):
            wc = load_wcol(nc, wpool3, w_par["wo"], nt)
            xq2 = xq2pool.tile([128, MQ], f32, name="xq2", tag="xq2")
            nc.gpsimd.dma_start(out=xq2, in_=xq2_view[:, nt, :])
            for qc in range(QC):
                qs = slice(qc * 512, (qc + 1) * 512)
                ps = p3ps.tile([128, 512], f32, name="ps_o", tag="p3")
                for kt in range(NT):
                    nc.tensor.matmul(
                        ps, wc[:, kt, :], ctxT[:, kt, qs],
                        start=(kt == 0), stop=(kt == NT - 1))
                nc.vector.tensor_scalar(
                    out=y1[:, nt, qs], in0=ps,
                    scalar1=bias_sb["bo"][:, nt:nt + 1], scalar2=None,
                    op0=mybir.AluOpType.add)
                nc.vector.tensor_add(out=y1[:, nt, qs], in0=y1[:, nt, qs],
                                     in1=xq2[:, qs])
    ctxT_free()

    # ======================= P4: LayerNorm 1 =================================
    n1, n1_free = tc.tile([128, NT, MQ], F32R, name="n1", side="right")
    layernorm(nc, tc, src=y1, dst=n1, g_sb=bias_sb["g1"], b_sb=bias_sb["b1"],
              ones_sb=ones_sb, onesr_sb=onesr_sb, eps_sb=eps_sb, label="ln1")
    y1_free()

    # ======================= P5: FFN ==========================================
    fnn, fnn_free = tc.tile([128, NT, MQ], F32R, name="fnn", side="left")
    with tc.tile_pool(name="wpool5", bufs=3, side="left") as wpool5, \
         tc.tile_pool(name="p5ps", bufs=4, space="PSUM") as p5ps:
        for nt in range(NT):
            wc = load_wcol(nc, wpool5, w_par["wf"], nt)
            for qc in range(QC):
                qs = slice(qc * 512, (qc + 1) * 512)
                ps = p5ps.tile([128, 512], f32, name="ps_f", tag="p5")
                for kt in range(NT):
                    nc.tensor.matmul(
                        ps, wc[:, kt, :], n1[:, kt, qs],
                        start=(kt == 0), stop=(kt == NT - 1))
                nc.scalar.activation(
                    out=fnn[:, nt, qs], in_=ps, func=AF.Relu,
                    bias=bias_sb["bf"][:, nt:nt + 1], scale=1.0)

    # y2 = fnn + n1 (overwrite fnn)
    for qc in range(QC):
        qs = slice(qc * 512, (qc + 1) * 512)
        for nt in range(NT):
            nc.vector.tensor_add(out=fnn[:, nt, qs], in0=fnn[:, nt, qs],
                                 in1=n1[:, nt, qs])
    n1_free()

    # ======================= P6: LayerNorm 2 -> output =======================
    with tc.tile_pool(name="outpool", bufs=3, side="left") as outpool:
        ab = ln_stats(nc, tc, src=fnn, ones_sb=ones_sb, onesr_sb=onesr_sb,
                      eps_sb=eps_sb, label="ln2")
        for qc in range(QC):
            qs = slice(qc * 512, (qc + 1) * 512)
            for nt in range(NT):
                n2t = outpool.tile([128, 512], f32, name="n2t", tag="n2")
                apply_ln(nc, n2t, fnn[:, nt, qs], ab, qc,
                         bias_sb["g2"][:, nt:nt + 1], bias_sb["b2"][:, nt:nt + 1])
                nc.sync.dma_start(out=outT[nt * 128:(nt + 1) * 128, qs], in_=n2t)
        ab[2]()
    fnn_free()

    for fr in reversed(final_frees):
        fr()
    return []


def ln_stats(nc, tc, src, ones_sb, onesr_sb, eps_sb, label):
    """column sums of src and src^2 over all D partitions via ones-matmuls;
    A=rstd and B=-mean*rstd are then PE-broadcast into PSUM tiles."""
    A, A_free = tc.tile([1, MQ], F32, name=f"A_{label}", side="right")
    Bt, B_free = tc.tile([1, MQ], F32, name=f"B_{label}", side="right")
    m_sb, m_free = tc.tile([1, MQ], F32, name=f"m_{label}", side="right")
    lnps = tc.alloc_tile_pool(name=f"lnps_{label}", bufs=2, space="PSUM")
    abps = tc.alloc_tile_pool(name=f"abps_{label}", bufs=2, space="PSUM")
    ab_tiles = []
    with tc.tile_pool(name=f"sq_{label}", bufs=2, side="left") as sqpool:
        for qc in range(QC):
            qs = slice(qc * 512, (qc + 1) * 512)
            ps_sum = lnps.tile([128, 512], F32, name="ps_sum", tag="lnsum")
            ps_sq = lnps.tile([128, 512], F32, name="ps_sq", tag="lnsum")
            for nt in range(NT):
                sq = sqpool.tile([128, 512], F32R, name="sq", tag="sq")
                # square on ScalarE (idle in the tail) instead of VectorE
                nc.scalar.activation(out=sq, in_=src[:, nt, qs], func=AF.Square)
                nc.tensor.matmul(ps_sum[0:1, :], ones_sb[:, :], src[:, nt, qs],
                                 start=(nt == 0), stop=(nt == NT - 1))
                nc.tensor.matmul(ps_sq[0:1, :], ones_sb[:, :], sq[:, :],
                                 start=(nt == 0), stop=(nt == NT - 1))
            # mean, var, A = 1/sqrt(var+eps), B = -mean*A  (on [1, 512])
            nc.scalar.mul(out=m_sb[0:1, qs], in_=ps_sum[0:1, :], mul=1.0 / D)
            nc.scalar.mul(out=A[0:1, qs], in_=ps_sq[0:1, :], mul=1.0 / D)
            nc.vector.tensor_mul(out=Bt[0:1, qs], in0=m_sb[0:1, qs],
                                 in1=m_sb[0:1, qs])
            nc.vector.tensor_sub(out=A[0:1, qs], in0=A[0:1, qs], in1=Bt[0:1, qs])
            nc.scalar.activation(out=A[0:1, qs], in_=A[0:1, qs], func=AF.Sqrt,
                                 bias=eps_sb[0:1, 0:1], scale=1.0)
            nc.vector.reciprocal(out=A[0:1, qs], in_=A[0:1, qs])
            nc.vector.tensor_mul(out=Bt[0:1, qs], in0=m_sb[0:1, qs],
                                 in1=A[0:1, qs])
            nc.scalar.mul(out=Bt[0:1, qs], in_=Bt[0:1, qs], mul=-1.0)
            # PE-broadcast A/B rows to all 128 partitions (into PSUM)
            A_ps = abps.tile([128, 512], F32, name="A_ps", tag="lnbc")
            B_ps = abps.tile([128, 512], F32, name="B_ps", tag="lnbc")
            nc.tensor.matmul(A_ps, onesr_sb[:, :], A[0:1, qs],
                             start=True, stop=True)
            nc.tensor.matmul(B_ps, onesr_sb[:, :], Bt[0:1, qs],
                             start=True, stop=True)
            ab_tiles.append((A_ps, B_ps))

    def frees():
        abps.release()
        lnps.release()
        m_free()
        B_free()
        A_free()

    return (ab_tiles, None, frees)


def apply_ln(nc, out_ap, y_ap, ab, qc, g_col, b_col):
    A_ps, B_ps = ab[0][qc]
    nc.vector.tensor_mul(out=out_ap, in0=y_ap, in1=A_ps)
    nc.vector.tensor_add(out=out_ap, in0=out_ap, in1=B_ps)
    nc.vector.tensor_scalar(
        out=out_ap, in0=out_ap, scalar1=g_col, scalar2=b_col,
        op0=mybir.AluOpType.mult, op1=mybir.AluOpType.add)


def layernorm(nc, tc, src, dst, g_sb, b_sb, ones_sb, onesr_sb, eps_sb, label):
    ab = ln_stats(nc, tc, src=src, ones_sb=ones_sb, onesr_sb=onesr_sb,
                  eps_sb=eps_sb, label=label)
    for qc in range(QC):
        qs = slice(qc * 512, (qc + 1) * 512)
        for nt in range(NT):
            apply_ln(nc, dst[:, nt, qs], src[:, nt, qs], ab, qc,
                     g_sb[:, nt:nt + 1], b_sb[:, nt:nt + 1])
    ab[2]()


_NC_CACHE = {}


def _get_nc():
    if "nc" not in _NC_CACHE:
        _NC_CACHE["nc"] = build_nc()
    return _NC_CACHE["nc"]


def kernel(x, wq, bq, wk, bk, wv, bv, wo, bo, wf, bf, g1, b1, g2, b2,
           _trace=False):
    nc = _get_nc()
    x = np.ascontiguousarray(np.asarray(x, dtype=np.float32))
    weights = {}
    for nm, v in (("wq", wq), ("wk", wk), ("wv", wv), ("wo", wo), ("wf", wf),
                  ("bq", bq), ("bk", bk), ("bv", bv), ("bo", bo), ("bf", bf),
                  ("g1", g1), ("b1", b1), ("g2", g2), ("b2", b2)):
        weights[nm] = np.ascontiguousarray(np.asarray(v, dtype=np.float32))
    in_maps = []
    for c in range(8):
        b, half = divmod(c, 2)
        xT = np.ascontiguousarray(x[b].T)                              # [D, S]
        xqTc = np.ascontiguousarray(x[b, half * MQ:(half + 1) * MQ].T)  # [D, MQ]
        in_maps.append({"xkvT": xT, "xqT": xqTc, **weights})
    res = run_bass_kernel_spmd(nc, in_maps, core_ids=list(range(8)), trace=_trace)
    out = np.empty((B, S, D), dtype=np.float32)
    for c in range(8):
        b, half = divmod(c, 2)
        out[b, half * MQ:(half + 1) * MQ, :] = res.results[c]["outT"].T
    if _trace:
        return out, res
    return out
